# revision 40
# baseline (speedup 1.0000x reference)
"""Block-diagonal grouped GEMM (BlockDense) for Trainium2, 8 NeuronCores.

Problem: x:(8192, 16384) f32, W:(1024, 16, 16) f32
         out[b, g*16+h] = relu(sum_w x[b, g*16+w] * W[g, w, h])

Strategy:
  - Data-parallel shard of the batch dim across 8 cores (1024 rows each).
  - Memory-regime problem (0.5 GB in + 0.5 GB out, tiny compute): cast
    x/W/out to fp16 on the host, halving HBM traffic per core from
    ~129 MB to ~64.5 MB. fp16 keeps ~4e-4 rel err (10 mantissa bits),
    and the PE runs fp16 at 1 cycle/row vs fp32's 4.
  - Host relayouts each x shard so features sit on SBUF partitions
    (the PE contracts along partitions); 8 groups are packed into one
    128x128 block-diagonal weight supergroup so the full PE array is used.
  - The weights are the STATIONARY matmul operand; 512 batch columns
    stream per matmul. 512-row matmuls amortize the ~173 ns PE SBUF
    access latency that dominated 128-row matmuls (283 ns each -> the
    PE, not DMA, paced the kernel). Output therefore lands transposed
    (out-col on partitions, batch on free dim); the host un-transposes.
  - Per core: 16 column blocks (8 supergroups each): DMA the 2 MB
    x-block, per supergroup LDW + 2 matmuls (512 batch) into PSUM,
    relu PSUM->SBUF(fp16) on alternating Scalar/Vector engines, and
    one 1 MB store (8 KB runs) per 4 supergroups.
"""

import sys

import numpy as np

import concourse.bass as bass
import concourse.mybir as mybir
import concourse.tile as tile
from concourse import bacc, bass_utils
from concourse.tile_rust import add_dep_helper


def _ensure_axon_hooks_shim():
    """The bare agent image lacks antenv.axon_hooks; bass_utils imports it
    when trace=True under axon. Provide a working shim (ctypes NTFF hook if
    the axon .so supports it, else None -> tracing is skipped gracefully)."""
    try:
        import antenv.axon_hooks  # noqa: F401
        return
    except ImportError:
        pass
    import types

    hook = None
    try:
        from trn_agent_boot.trn_boot import _ntff_profile_via_ctypes

        hook = _ntff_profile_via_ctypes("/opt/axon/libaxon_pjrt.so")
    except Exception:
        hook = None
    mod = types.ModuleType("antenv.axon_hooks")
    mod.get_axon_ntff_profile_hook = lambda: hook
    mod.set_axon_ntff_profile_hook = lambda h: None
    try:
        import antenv

        antenv.axon_hooks = mod
    except ImportError:
        pass
    sys.modules["antenv.axon_hooks"] = mod


_ensure_axon_hooks_shim()

# Problem constants (hardcoded per contract; kernel.py must be self-contained)
G, W_SZ, H = 1024, 16, 16
B = 8192
F = G * W_SZ  # 16384 input features = output features (H == W_SZ)
N_CORES = 8
B_LOC = B // N_CORES  # 1024 batch rows per core

P = 128          # partitions
GROUPS_PER_SG = 128 // W_SZ   # 8 groups per 128x128 supergroup
N_SG = G // GROUPS_PER_SG     # 128 supergroups
SG_PER_BLK = 8                # supergroups per column block
N_BLK = N_SG // SG_PER_BLK    # 16 column blocks of 1024 columns
MM_ROWS = 512                 # moving rows per matmul (one PSUM bank)
MM_PER_SG = B_LOC // MM_ROWS  # 2 matmuls per supergroup

_cached = {}

# experiment knobs (bench only; defaults are the shipping config)
CONFIG = {
    "out_engine": "scalar",  # sync | scalar  (which HWDGE ring issues stores)
    "x_bufs": 8,             # x block tiles resident
    "o_bufs": 5,             # 5x8KB fits beside x_bufs=8 (200KB of 208KB)
    "relu_mix": "alt",       # alt | act | dve
    "sg_per_store": 4,       # supergroups per output store (4 -> 1MB/8KB runs)
    "serial_x": 0,           # 1: chain x loads (adds ~2us/load sem latency)
    "first_split": 4,        # pieces for the first 2 block loads
    "last_split": 4,         # pieces for the last block load (shrinks tail)
    "fuse_mm": 0,            # 1024-row matmul/sg fails ISA (1 bank max); keep 0
    "defer_store": 1,        # issue each store one group late: its relu waits
                             # are satisfied by then, so it doesn't head-of-
                             # line-block the next group's relus on the ACT
                             # queue (which stalled the PE ~1.4us per group)
}


def _build_program():
    """Build the (single-core SPMD) bass program once per process."""
    key = tuple(sorted(CONFIG.items()))
    if key in _cached:
        return _cached[key]

    f32 = mybir.dt.float32
    f16 = mybir.dt.float16
    nc = bacc.Bacc("TRN2", debug=False, target_bir_lowering=False)

    xt_d = nc.dram_tensor("xt", (N_BLK, P, SG_PER_BLK * B_LOC), f16,
                          kind="ExternalInput")
    # pre-expanded block-diagonal weights, supergroup-major (4 MB fp16):
    #   wt[i, sg*128 + (jj*16 + h)] = W[8*sg + jj, w, h]   (i = 16jj+w)
    # so the stationary AP for sg is one CONTIGUOUS 128-col slice (LDW
    # 137 ns vs 250 ns strided; on-chip expansion of the compact 0.5 MB
    # form costs more than the extra 3.5 MB of HBM reads).
    wt_d = nc.dram_tensor("wt", (P, N_SG * P), f16, kind="ExternalInput")
    # transposed output: out_t[p, sg*1024 + b] = out[b, sg*128 + p]
    out_d = nc.dram_tensor("out", (P, N_SG * B_LOC), f16,
                           kind="ExternalOutput")

    xt_ap = xt_d.ap()
    wt_ap = wt_d.ap()
    out_ap = out_d.ap()

    relu = mybir.ActivationFunctionType.Relu

    out_dma = nc.scalar if CONFIG["out_engine"] == "scalar" else nc.sync

    SPS = CONFIG["sg_per_store"]   # supergroups per store

    W_CH = 8                      # weight tile loaded in 8 x 512KB chunks
    SG_PER_CH = N_SG // W_CH      # 16 supergroups per chunk

    with tile.TileContext(nc) as tc:
        with (
            tc.tile_pool(name="w2pool", bufs=1) as w2pool,
            tc.tile_pool(name="xpool", bufs=CONFIG["x_bufs"]) as xpool,
            tc.tile_pool(name="opool", bufs=CONFIG["o_bufs"]) as opool,
            tc.tile_pool(name="pspool", bufs=4,
                         space=bass.MemorySpace.PSUM) as pspool,
        ):
            wt2 = w2pool.tile([P, N_SG * P], f16)

            # All weight chunks ride the STORE (scalar) HWDGE ring, issued
            # up front: that ring is idle until the first store (~22us), so
            # the 4MB of weights stops competing with the x stream on the
            # sync ring, and no store precedes them in the queue (no
            # head-of-line risk — chunk DMAs have no waits).
            for c in range(W_CH):
                lo = c * SG_PER_CH * P
                hi = (c + 1) * SG_PER_CH * P
                out_dma.dma_start(wt2[:, lo:hi], wt_ap[:, lo:hi])

            wt_sg = wt2[:].rearrange("p (sg o) -> p sg o", o=P)

            prev_load = [None]

            def load_x(blk):
                xt_t = xpool.tile([P, SG_PER_BLK * B_LOC], f16)
                # finer pieces for the first loads so compute starts sooner,
                # and for the last load so the tail drains sooner
                if blk < 2:
                    nsp = CONFIG["first_split"]
                elif blk == N_BLK - 1:
                    nsp = CONFIG["last_split"]
                else:
                    nsp = 1
                piece = (SG_PER_BLK * B_LOC) // nsp
                for sp in range(nsp):
                    di = nc.sync.dma_start(
                        xt_t[:, sp * piece:(sp + 1) * piece],
                        xt_ap[blk, :, sp * piece:(sp + 1) * piece],
                    )
                    if CONFIG["serial_x"]:
                        if prev_load[0] is not None:
                            add_dep_helper(di.ins, prev_load[0],
                                           reason="serialize x loads")
                        prev_load[0] = di.ins
                return xt_t

            mix = CONFIG["relu_mix"]

            def do_relu(dst, src_ps, idx):
                use_act = (mix == "act" or (mix == "alt" and idx % 2 == 0))
                if use_act:
                    nc.scalar.activation(dst, src_ps, relu)
                else:
                    nc.vector.tensor_scalar_max(dst, src_ps, 0.0)

            pending_store = []

            def flush_store():
                ot, sg0 = pending_store.pop(0)
                if sg0 >= N_SG - 2 * SPS:
                    # final stores: two halves so the tail drains finer
                    half_cols = SPS * B_LOC // 2
                    for hs in range(2):
                        out_dma.dma_start(
                            out_ap[:, sg0 * B_LOC + hs * half_cols:
                                   sg0 * B_LOC + (hs + 1) * half_cols],
                            ot[:, hs * half_cols:(hs + 1) * half_cols],
                        )
                else:
                    out_dma.dma_start(
                        out_ap[:, sg0 * B_LOC:(sg0 + SPS) * B_LOC],
                        ot[:],
                    )

            for blk in range(N_BLK):
                xt_t = load_x(blk)
                for hh in range(SG_PER_BLK // SPS):
                    ot = opool.tile([P, SPS * B_LOC], f16)
                    for u in range(SPS):
                        j = hh * SPS + u
                        sg = blk * SG_PER_BLK + j
                        lhsT = wt_sg[:, sg, :]
                        # two 512-row matmuls (1 PSUM bank each, the ISA
                        # cap) into one 2-bank tile, drained by ONE relu:
                        # [128, 1024] relu amortizes the ~320 ns engine
                        # overhead that made 512-col relus (687 ns each,
                        # 427 ns/tile across 2 engines) pace the PE, which
                        # runs 512-row matmuls every 216 ns when unstalled.
                        ps = pspool.tile([P, B_LOC], f32)
                        for half in range(MM_PER_SG):
                            rhs = xt_t[:, j * B_LOC + half * MM_ROWS:
                                       j * B_LOC + (half + 1) * MM_ROWS]
                            nc.tensor.matmul(
                                ps[:, half * MM_ROWS:(half + 1) * MM_ROWS],
                                lhsT, rhs, start=True, stop=True)
                        do_relu(ot[:, u * B_LOC:(u + 1) * B_LOC], ps[:], u)
                    sg0 = blk * SG_PER_BLK + hh * SPS
                    pending_store.append((ot, sg0))
                    # deferral trades head-of-line queue blocking for tail
                    # latency; on the last block nothing follows, so flush
                    # immediately to shorten the tail.
                    defer = CONFIG["defer_store"] if blk < N_BLK - 1 else 0
                    while len(pending_store) > defer:
                        flush_store()
            while pending_store:
                flush_store()

    nc.compile()
    _cached[key] = nc
    return nc


def _prep_w(W: np.ndarray) -> np.ndarray:
    """Pre-expanded block-diagonal fp16 weights, supergroup-major:

    wt[16*jj + w, sg*128 + jj*16 + h] = W[8*sg + jj, w, h]
    """
    Wr = np.ascontiguousarray(W, dtype=np.float32).reshape(
        N_SG, GROUPS_PER_SG, W_SZ, H).astype(np.float16)   # [sg, jj, w, h]
    wt = np.zeros((GROUPS_PER_SG, W_SZ, N_SG, GROUPS_PER_SG, H),
                  dtype=np.float16)                         # [jj, w, sg, jj2, h]
    for jj in range(GROUPS_PER_SG):
        wt[jj, :, :, jj, :] = Wr[:, jj].transpose(1, 0, 2)  # [w, sg, h]
    return np.ascontiguousarray(wt.reshape(P, N_SG * P))


def _prep_x_shard(xs: np.ndarray) -> np.ndarray:
    """Relayout one (1024, 16384) fp16 shard to (16, 128, 8*1024).

    xt[blk, p, j*1024 + b] = xs[b, blk*1024 + j*128 + p]
    """
    x4 = xs.reshape(B_LOC, N_BLK, SG_PER_BLK, P)          # b, blk, j, p
    xt = np.ascontiguousarray(x4.transpose(1, 3, 2, 0))    # blk, p, j, b
    return xt.reshape(N_BLK, P, SG_PER_BLK * B_LOC)


# Debug/benchmark knobs (used by test.py only; harness leaves defaults)
TRACE = False
TRACE_CORES = None  # e.g. [0] or list(range(8))
LAST_RESULTS = None


def kernel(x: np.ndarray, W: np.ndarray) -> np.ndarray:
    global LAST_RESULTS
    assert x.shape == (B, F) and W.shape == (G, W_SZ, H)
    x16 = np.ascontiguousarray(x, dtype=np.float32).astype(np.float16)

    wt = _prep_w(W)
    in_maps = []
    for s in range(N_CORES):
        xs = x16[s * B_LOC:(s + 1) * B_LOC]
        in_maps.append({"xt": _prep_x_shard(xs), "wt": wt})

    nc = _build_program()
    kwargs = {}
    if TRACE:
        kwargs = {"trace": True, "trace_cores": TRACE_CORES}
    res = bass_utils.run_bass_kernel_spmd(nc, in_maps,
                                          core_ids=list(range(N_CORES)),
                                          **kwargs)
    LAST_RESULTS = res
    out = np.empty((B, F), dtype=np.float32)
    for s, r in enumerate(res.results):
        # out_t[p, sg*1024 + b] = out[b, sg*128 + p]
        ot = r["out"].reshape(P, N_SG, B_LOC)
        out[s * B_LOC:(s + 1) * B_LOC] = (
            ot.transpose(2, 1, 0).reshape(B_LOC, F).astype(np.float32))
    return out
